# revision 3
# baseline (speedup 1.0000x reference)
"""Multi-head attention (B=2, S=2048, D=1024, H=16) on 8 TRN2 cores.

Sharding (sequence-parallel): core c -> batch b = c//4, q-token shard
r = c%4 (tokens 512r..512r+511). Every core projects the FULL k/v for its
batch (4x redundant) and computes all 16 heads for its 512 q tokens, so the
output projection is fully local -- no inter-core collective.

v2 changes vs the 205us baseline:
  * all matmul operands fp16 (was bf16): rel err 0.0065 -> ~0.001, same speed.
  * QK row-tiling: per head, the two sk tiles of a pair run as TWO CONCURRENT
    K=64 matmuls on PE row-tiles 0/1 (tile_position (0,0)/(64,0) auto-derived
    from base partitions). k lives interleaved: k_both[0:64, h, j] = depth dims
    of head h for sk tile 2j, k_both[64:128, h, j] = tile 2j+1 (built with two
    partition-shift SBUF DMAs per k-proj psum). q lives duplicated into both
    partition halves (q_dup, built with shift DMAs off the q-proj psum).
    This halves QK's PE time (K=64 wasted half the 128-deep array).
  * the K=65 mask-bias fold is gone: masked keys are host-compacted away as
    before, and PADDED keys (to the 128 multiple) are handled exactly by
    zeroing their xk columns (logit 0, exp = 1) AND their v rows and aug
    (ones) rows, so pads contribute 0 to both numerator and denominator.
  * exp needs no bias: one activation per TWO sk tiles over the 2-bank psl.

Structure is otherwise the baseline's: per-head pipeline QK pair -> exp ->
AV pair (lag 1), deferred normalize (reciprocal + K=1 ones matmul broadcast)
and q-proj blocks 1..7 drip-fed into the attention loop, local out-proj.
"""

import numpy as np
from contextlib import ExitStack

import concourse.bass as bass
import concourse.tile as tile
from concourse import mybir
from concourse._compat import with_exitstack

F32 = mybir.dt.float32
F16 = mybir.dt.float16
AF = mybir.ActivationFunctionType
F16_NP = np.float16


B, S, D = 2, 2048, 1024
NCORES = 8
NH = 16                  # heads per core (all of them)
DH = 64
SQ = 512                 # q tokens per core
SKT = 128                # sk tile
NKT = D // 128           # 8 contraction/output 128-blocks
NAUG = 4                 # ones columns per head
VW = DH + NAUG           # 68: AV rows 64..67 = softmax denominator
SCALE = 0.125            # 1/sqrt(64)


@with_exitstack
def _mha(ctx: ExitStack, tc: "tile.TileContext", nsk, out, xq, xk, xv,
         wq, wk, wv, wo, aug, oneb):
    nc = tc.nc
    P = 128
    KP = nsk * SKT       # padded compacted key-token count
    NPAIR = (nsk + 1) // 2

    persist = ctx.enter_context(tc.tile_pool(name="persist", bufs=1))

    def T(shape, name, dt=F16):
        return persist.tile(shape, dt, name=name, tag=name)

    wq_sb = T([P, NKT * D], "wq_sb")
    wk_sb = T([P, NKT * D], "wk_sb")
    wv_sb = T([P, NKT * D], "wv_sb")
    wo_sb = T([P, NKT * D], "wo_sb")
    xq_sb = T([P, NKT * SQ], "xq_sb")
    # q duplicated into both partition halves; k interleaved by sk parity:
    # rows 0:64 = head h depth dims for even sk tiles, 64:128 = odd sk tiles.
    q_dup = T([P, NH, SQ], "q_dup")
    k_both = T([P, NH, NPAIR, SKT], "k_both")
    v_sb = T([P, nsk, NH, VW], "v_sb")
    at4 = T([P, NKT * SQ], "at4")
    aug_sb = T([P, nsk, NH, NAUG], "aug_sb")
    ones_sb = T([1, DH], "ones_sb")

    # ---- phase 1 scoped x-input staging (SBUF freed before attention) ----
    xin_pool = tc.tile_pool(name="xinp", bufs=1)
    with xin_pool as xin_p, \
         tc.tile_pool(name="ppk", bufs=4, space="PSUM") as ppk, \
         tc.tile_pool(name="ppv", bufs=2, space="PSUM") as ppv, \
         tc.tile_pool(name="kstg", bufs=3) as kstg:
        xk_sb = xin_p.tile([P, NKT * KP], F16, name="xk_sb", tag="xk_sb")
        xv_sb = xin_p.tile([P, NKT * KP], F16, name="xv_sb", tag="xv_sb")

        # weight/x streams: wk col-halves first (k-proj blocks 0-3 start
        # after 1MB instead of 2MB), then xk, wk second halves, wv, xv,
        # q inputs; tiny aug/ones rows after; wo arrives during attention.
        H2 = D // 2
        for k in range(NKT):
            nc.sync.dma_start(
                wk_sb[:, bass.ds(k * D, H2)], wk[bass.ts(k, P), 0:H2])
        for k in range(NKT):
            nc.sync.dma_start(xk_sb[:, bass.ts(k, KP)], xk[bass.ts(k, P), :])
        for k in range(NKT):
            nc.sync.dma_start(
                wk_sb[:, bass.ds(k * D + H2, H2)], wk[bass.ts(k, P), H2:D])
        for k in range(NKT):
            nc.sync.dma_start(wv_sb[:, bass.ts(k, D)], wv[bass.ts(k, P), :])
        for k in range(NKT):
            nc.sync.dma_start(xv_sb[:, bass.ts(k, KP)], xv[bass.ts(k, P), :])
        for k in range(NKT):
            nc.sync.dma_start(wq_sb[:, bass.ts(k, D)], wq[bass.ts(k, P), :])
        for k in range(NKT):
            nc.sync.dma_start(xq_sb[:, bass.ts(k, SQ)], xq[bass.ts(k, P), :])
        nc.sync.dma_start(aug_sb[:, :, :, :], aug[:, :, :, :])
        nc.sync.dma_start(ones_sb[:], oneb[:, 0:DH])

        # ---- k projection + interleaved placement ----
        # psum ps[128, csz]: rows 0:64 = head 2m, 64:128 = head 2m+1 over
        # csz consecutive compacted key tokens (4 sk tiles per 512 chunk).
        for m in range(NKT):
            for tc0 in range(0, KP, SQ):
                csz = min(SQ, KP - tc0)
                nt = csz // SKT                       # sk tiles this chunk
                t0 = tc0 // SKT
                ps = ppk.tile([P, SQ // SKT, SKT], F32, name="ps")
                for k in range(NKT):
                    nc.tensor.matmul(
                        ps[:, 0:nt, :],
                        lhsT=wk_sb[:, bass.ds(k * D + m * P, P)],
                        rhs=xk_sb[:, bass.ds(k * KP + tc0, csz)],
                        start=(k == 0),
                        stop=(k == NKT - 1),
                    )
                # aligned copies: even tiles of head 2m, odd tiles of 2m+1
                ne = (nt + 1) // 2                    # even-parity tiles here
                no = nt // 2                          # odd-parity tiles
                j0 = t0 // 2
                nc.vector.tensor_copy(
                    k_both[bass.ds(0, DH), 2 * m, j0:j0 + ne, :],
                    ps[bass.ds(0, DH), bass.ds(0, ne, 2), :],
                )
                if no:
                    nc.vector.tensor_copy(
                        k_both[bass.ds(DH, DH), 2 * m + 1, j0:j0 + no, :],
                        ps[bass.ds(DH, DH), bass.ds(1, no, 2), :],
                    )
                # shifted halves stage to fp16 then SBUF-DMA across partitions
                kt = kstg.tile([P, 2, SKT], F16, name="kt")
                if no:
                    nc.vector.tensor_copy(
                        kt[bass.ds(0, DH), 0:no, :],
                        ps[bass.ds(0, DH), bass.ds(1, no, 2), :],
                    )
                    nc.sync.dma_start(
                        k_both[bass.ds(DH, DH), 2 * m, j0:j0 + no, :],
                        kt[bass.ds(0, DH), 0:no, :],
                    )
                nc.vector.tensor_copy(
                    kt[bass.ds(DH, DH), 0:ne, :],
                    ps[bass.ds(DH, DH), bass.ds(0, ne, 2), :],
                )
                nc.sync.dma_start(
                    k_both[bass.ds(0, DH), 2 * m + 1, j0:j0 + ne, :],
                    kt[bass.ds(DH, DH), 0:ne, :],
                )

        # ---- v projection (token-major): v_sb[tok 128, st, h, 68] ----
        for st in range(nsk):
            for half in range(2):
                hh = NH // 2
                psv = ppv.tile([P, hh, DH], F32, name="psv")
                for k in range(NKT):
                    nc.tensor.matmul(
                        psv[:, :, :],
                        lhsT=xv_sb[:, bass.ds(k * KP + st * SKT, SKT)],
                        rhs=wv_sb[:, bass.ds(k * D + half * hh * DH, hh * DH)],
                        start=(k == 0),
                        stop=(k == NKT - 1),
                    )
                nc.vector.tensor_copy(
                    v_sb[:, st, half * hh:(half + 1) * hh, 0:DH], psv[:, :, :]
                )
                nc.vector.tensor_copy(
                    v_sb[:, st, half * hh:(half + 1) * hh, DH:VW],
                    aug_sb[:, st, half * hh:(half + 1) * hh, :],
                )

        # wo during attention
        for k in range(NKT):
            nc.sync.dma_start(wo_sb[:, bass.ts(k, D)], wo[bass.ts(k, P), :])

    # ---- attention: 16 heads x nsk sk-tiles over this core's 512 q ----
    with tc.tile_pool(name="expp", bufs=3) as exp_pool, \
         tc.tile_pool(name="pslp", bufs=2, space="PSUM") as psl_pool, \
         tc.tile_pool(name="psop", bufs=2, space="PSUM") as pso_pool, \
         tc.tile_pool(name="pbp", bufs=1, space="PSUM") as pb_pool, \
         tc.tile_pool(name="qpp", bufs=1, space="PSUM") as qp_pool, \
         tc.tile_pool(name="qstg", bufs=2) as qstg, \
         tc.tile_pool(name="recp", bufs=4) as rec_pool:

        norm_q = []
        misc_q = []

        def drain(q):
            if q:
                q.pop(0)()

        def make_qproj(m):
            def fn():
                ps = qp_pool.tile([P, SQ], F32, name="qp")
                for k in range(NKT):
                    nc.tensor.matmul(
                        ps[:],
                        lhsT=wq_sb[:, bass.ds(k * D + m * P, P)],
                        rhs=xq_sb[:, bass.ts(k, SQ)],
                        start=(k == 0),
                        stop=(k == NKT - 1),
                    )
                qt = qstg.tile([P, SQ], F16, name="qt")
                nc.vector.tensor_copy(qt[:], ps[:])
                nc.sync.dma_start(q_dup[bass.ds(0, DH), 2 * m, :], qt[bass.ds(0, DH), :])
                nc.sync.dma_start(q_dup[bass.ds(DH, DH), 2 * m, :], qt[bass.ds(0, DH), :])
                nc.sync.dma_start(q_dup[bass.ds(0, DH), 2 * m + 1, :], qt[bass.ds(DH, DH), :])
                nc.sync.dma_start(q_dup[bass.ds(DH, DH), 2 * m + 1, :], qt[bass.ds(DH, DH), :])
            return fn

        def make_normalize(h, pso):
            pr, po = h // 2, (h % 2) * DH

            def fn():
                den1 = rec_pool.tile([1, SQ], F32, name="den1")
                nc.vector.tensor_copy(den1[:], pso[bass.ds(DH, 1), :])
                rec_f = rec_pool.tile([1, SQ], F32, name="rec_f")
                nc.vector.reciprocal_approx_fast(rec_f[:], den1[:])
                rec_b = rec_pool.tile([1, SQ], F16, name="rec_b")
                nc.vector.tensor_copy(rec_b[:], rec_f[:])
                pb = pb_pool.tile([DH, SQ], F32, name="pb")
                nc.tensor.matmul(
                    pb[:], lhsT=ones_sb[:], rhs=rec_b[:], start=True, stop=True
                )
                dst = at4[bass.ds(po, DH), bass.ds(pr * SQ, SQ)]
                nc.vector.tensor_copy(dst, pso[bass.ds(0, DH), :])
                nc.vector.tensor_mul(dst, dst, pb[:])

            return fn

        # q block 0 inline (heads 0/1 need it first); 1..7 drip-fed two per
        # head so block m lands well before heads 2m/2m+1 need it.
        make_qproj(0)()
        misc_q.extend(make_qproj(m) for m in range(1, NKT))

        # global lag-1 pipeline over the flat (head, pair) sequence: the
        # next head's first QK is emitted before the previous head's last
        # AV pair, so the ACT engine never idles at head boundaries.
        pairs = [tuple(range(j, min(j + 2, nsk))) for j in range(0, nsk, 2)]
        psos = {}

        def emit_av(ex_t, h, pair):
            for i, sk_i in enumerate(pair):
                nc.tensor.matmul(
                    psos[h][:],
                    lhsT=v_sb[:, sk_i, h, :],
                    rhs=ex_t[:, bass.ts(i, SQ)],
                    start=(sk_i == 0),
                    stop=(sk_i == nsk - 1),
                    skip_group_check=True,
                )
            if pair[-1] == nsk - 1:
                norm_q.append(make_normalize(h, psos.pop(h)))

        prev = None
        for h in range(NH):
            # robustness for small nsk (few in-loop drain slots): keep the
            # pso ring from being overrun and q blocks ahead of their heads.
            # Both loops are no-ops for nsk=8.
            while len(norm_q) > 1:
                drain(norm_q)
            while len(misc_q) > NKT - 1 - h // 2:
                drain(misc_q)
            psos[h] = pso_pool.tile([VW, SQ], F32, name="pso")
            for j, pair in enumerate(pairs):
                w = len(pair) * SQ
                psl = psl_pool.tile([P, 2 * SQ], F32, name="psl")
                # two concurrent K=64 row-tile matmuls: even sk tile from
                # partitions 0:64, odd from 64:128 (psl banks a / b).
                nc.tensor.matmul(
                    psl[:, 0:SQ],
                    lhsT=k_both[bass.ds(0, DH), h, j, :],
                    rhs=q_dup[bass.ds(0, DH), h, :],
                    start=True,
                    stop=True,
                )
                if len(pair) == 2:
                    nc.tensor.matmul(
                        psl[:, bass.ds(SQ, SQ)],
                        lhsT=k_both[bass.ds(DH, DH), h, j, :],
                        rhs=q_dup[bass.ds(DH, DH), h, :],
                        start=True,
                        stop=True,
                    )
                ex = exp_pool.tile([P, 2 * SQ], F16, name="ex")
                nc.scalar.activation(
                    ex[:, 0:w], psl[:, 0:w], AF.Exp, scale=SCALE,
                )
                if prev is not None:
                    emit_av(*prev)
                    # j==1 (the head-boundary window) stays drain-free so
                    # the next exp is never delayed by pb/qproj PE work.
                    if j == 2:
                        drain(norm_q)
                    elif j == 3:
                        drain(misc_q)
                prev = (ex, h, pair)
        emit_av(*prev)

        while norm_q or misc_q:
            drain(norm_q)
            drain(misc_q)

    # ---- local output projection (no collective) ----
    with tc.tile_pool(name="psfp", bufs=2, space="PSUM") as psf_pool, \
         tc.tile_pool(name="finp", bufs=2) as fin_pool:
        for m in range(NKT):
            psf = psf_pool.tile([P, SQ], F32, name="psf")
            for kt in range(NKT):
                nc.tensor.matmul(
                    psf[:],
                    lhsT=wo_sb[:, bass.ds(kt * D + m * P, P)],
                    rhs=at4[:, bass.ts(kt, SQ)],
                    start=(kt == 0),
                    stop=(kt == NKT - 1),
                )
            ot = fin_pool.tile([P, SQ], F16, name="ot")
            nc.vector.tensor_copy(ot[:], psf[:])
            nc.sync.dma_start(out[bass.ts(m, P), :], ot[:])


def build_program(nsk):
    from concourse import bacc

    KP = nsk * SKT
    nc = bacc.Bacc("TRN2", target_bir_lowering=False, debug=False, num_devices=NCORES)
    aps = {}
    for nm, shp, dt in (
        ("xq", [D, SQ], F16),
        ("xk", [D, KP], F16),
        ("xv", [D, KP], F16),
        ("wq", [D, D], F16),
        ("wk", [D, D], F16),
        ("wv", [D, D], F16),
        ("wo", [D, D], F16),
        ("aug", [128, nsk, NH, NAUG], F16),
        ("oneb", [1, DH], F16),
    ):
        aps[nm] = nc.dram_tensor(nm, shp, dt, kind="ExternalInput").ap()
    out = nc.dram_tensor("out", [D, SQ], F16, kind="ExternalOutput").ap()
    with tile.TileContext(nc) as tc:
        _mha(tc, nsk, out, **aps)
    nc.finalize()
    return nc


_NC_CACHE = {}


def _get_program(nsk):
    if nsk not in _NC_CACHE:
        _NC_CACHE[nsk] = build_program(nsk)
    return _NC_CACHE[nsk]


def pick_nsk(mask):
    n = max(int((mask[b] == 0).sum()) for b in range(B))
    return max(1, min(S // SKT, -(-n // SKT)))


def make_in_maps(nsk, query, key, value, mask, Wq, Wk, Wv, Wo):
    KP = nsk * SKT
    xkc, xvc, augs = {}, {}, {}
    for b in range(B):
        keep = np.flatnonzero(mask[b] == 0)[:KP]
        nk = len(keep)
        xk_b = np.zeros((D, KP), np.float32)
        xv_b = np.zeros((D, KP), np.float32)
        xk_b[:, :nk] = key[b].T[:, keep]
        xv_b[:, :nk] = value[b].T[:, keep]
        xkc[b] = xk_b.astype(F16_NP)
        xvc[b] = xv_b.astype(F16_NP)
        # aug ones-columns: zero for padded key slots (kills them in both
        # the AV numerator rows and the denominator row exactly).
        aug_flags = np.zeros((KP,), np.float16)
        aug_flags[:nk] = 1.0
        augs[b] = np.ascontiguousarray(
            np.broadcast_to(
                aug_flags.reshape(nsk, SKT).T[:, :, None, None],
                (SKT, nsk, NH, NAUG),
            )
        )
    wqT = Wq.T.astype(F16_NP)
    wkT = Wk.T.astype(F16_NP)
    wvT = Wv.T.astype(F16_NP)
    woT = Wo.T.astype(F16_NP)
    oneb = np.ones((1, DH), F16_NP)
    in_maps = []
    for c in range(NCORES):
        b, r = divmod(c, NCORES // B)
        in_maps.append(
            {
                "xq": np.ascontiguousarray(
                    query[b].T[:, r * SQ:(r + 1) * SQ]).astype(F16_NP),
                "xk": xkc[b],
                "xv": xvc[b],
                "wq": wqT,
                "wk": wkT,
                "wv": wvT,
                "wo": woT,
                "aug": augs[b],
                "oneb": oneb,
            }
        )
    return in_maps


def assemble_output(results):
    out = np.empty((B, S, D), dtype=np.float32)
    for c in range(NCORES):
        b, r = divmod(c, NCORES // B)
        out[b, r * SQ:(r + 1) * SQ, :] = results[c]["out"].astype(np.float32).T
    return out


def kernel(query, key, value, mask, Wq, bq, Wk, bk, Wv, bv, Wo, bo, trace=False):
    from concourse.bass_utils import run_bass_kernel_spmd

    mask = np.asarray(mask)
    nsk = pick_nsk(mask)
    nc = _get_program(nsk)
    in_maps = make_in_maps(
        nsk, np.asarray(query), np.asarray(key), np.asarray(value), mask,
        np.asarray(Wq), np.asarray(Wk), np.asarray(Wv), np.asarray(Wo),
    )
    br = run_bass_kernel_spmd(nc, in_maps, list(range(NCORES)), trace=trace)
    out = assemble_output(br.results)
    if trace:
        return out, br
    return out
